# revision 27
# baseline (speedup 1.0000x reference)
"""Trainium2 Bass kernel for nn_Node_GCN: out[n] = f(x[n]) + edge[n]^T @ g(cat(x,x)[n]).

Sharding: data-parallel over the batch dim N=8, one batch per NeuronCore.
Per core the dominant cost is streaming edge[n] from HBM once. edge and gx are
carried in fp8-e4m3 (fp32 PSUM accumulation), halving HBM traffic vs fp16 and
running DoubleRow matmuls (two 128-row contraction groups per instruction, 1
output column/cycle -> 2x fp16 FLOP rate). Edge rows are pair-interleaved via
the DMA access pattern (partition p holds rows 2p, 2p+1 of each 256-row block)
so descriptors are 4KB; the matching gx sender order comes from stride-2
stationary slices of h1g.

Timing model (measured): the NeuronCore runs at 1.2GHz until a HAM boost
event ~12us after the first PE matmul, then 2.4GHz. So: one tiny matmul
starts the boost timer immediately; everything else minimizes pre-boost PE
cycles. All loads ride one Sync-queue stream ordered [blob, edge x4] (a
parallel queue gets starved). Self-dynamics accumulates into pout (opens the
accumulation group) so the tail is just per-chunk PSUM->SBUF bias-copies
(ACT/DVE alternating) + two fp16 stores.

The device computes outT[n] = [h, j]; the host transposes to [j, h] while
unsharding.
"""

import numpy as np

D_IN = 64
D_HID = 128
M = 2048          # nodes per batch
N_BATCH = 8
NCORES = 8
NPAIR = M // 256  # 8 edge pair-blocks of 256 sender rows

# fp16 weights mini-blob [128, WM_W] (first DMA, unlocks the MLP chain
# early) + fp16 xT blob [128, 1024]. Rows 64:128 of the weight section
# duplicate rows 0:64 for the K=64 matmuls whose rhs lives on partitions
# 64:128 (xT packed [128, 1024]). fp32 biases are bit-packed as fp16 pairs.
_W_FW1 = 0          # f_w1 [64, 64]
_W_FW2 = 64         # f_w2 [64, 128]
_W_WG1 = 192        # wg1  [64, 128]  (= g_w1[:64] + g_w1[64:])
_W_GW2 = 320        # g_w2 [128, 128]
_W_GB2 = 448        # g_b2 broadcast rows, tiled 4x along free dim [128, 512]
_B_F1 = 960         # f_b1 [64, 1] fp32 as 2 fp16 slots
_B_G1 = 962         # g_b1 [128, 1] fp32 as 2 fp16 slots
_B_F2 = 964         # f_b2 [128, 1] fp32 as 2 fp16 slots
WM_W = 968

_NC_CACHE = {}


def _build():
    import concourse.bacc as bacc
    import concourse.mybir as mybir
    from concourse.tile import TileContext
    from concourse.bass import ts

    f32 = mybir.dt.float32
    f16 = mybir.dt.float16
    f8 = mybir.dt.float8e4
    AF = mybir.ActivationFunctionType
    DR = mybir.MatmulPerfMode.DoubleRow

    nc = bacc.Bacc()
    edge_d = nc.declare_dram_parameter("edge", [M, M], f8, isOutput=False)
    wm_d = nc.declare_dram_parameter("wm", [128, WM_W], f16, isOutput=False)
    xT_d = nc.declare_dram_parameter("xT", [128, M // 2], f16, isOutput=False)
    outT_d = nc.declare_dram_parameter("outT", [D_HID, M], f16, isOutput=True)

    NCH = M // 512   # 4 chunks of 512 for wide matmuls

    with TileContext(nc) as tc:
        with (
            tc.tile_pool(name="const", bufs=1) as cpool,
            tc.tile_pool(name="acts", bufs=1) as apool,
            tc.tile_pool(name="edgep", bufs=3) as epool,
            tc.tile_pool(name="pout", bufs=1, space="PSUM") as pout_pool,
            tc.tile_pool(name="pg", bufs=2, space="PSUM") as pg_pool,
            tc.tile_pool(name="pwork", bufs=2, space="PSUM") as pwork_pool,
        ):
            wm = cpool.tile([128, WM_W], f16, name="wm")
            xT = cpool.tile([128, M // 2], f16, name="xT")
            scratch = apool.tile([128, 512], f16, name="scratch")

            # Preamble loads first on the Sync queue (small loads lose
            # DMA-engine arbitration beside the edge stream): weights mini
            # blob, then xT, then the edge stream as 4 x 1MB, each covering
            # pair-blocks 2d, 2d+1.
            nc.sync.dma_start(out=wm, in_=wm_d[:])
            nc.sync.dma_start(out=xT[:, 0:512], in_=xT_d[:, 0:512])
            nc.sync.dma_start(out=xT[:, 512:1024], in_=xT_d[:, 512:1024])
            ets = []
            for b in range(NPAIR):
                et = epool.tile([128, 2, M], f8, tag="e", name=f"et{b}")
                src = edge_d[256 * b:256 * b + 256, :].rearrange(
                    "(p g) j -> p g j", g=2
                )
                nc.sync.dma_start(out=et, in_=src)
                ets.append(et)

            w_g2 = wm[0:128, _W_GW2:_W_GW2 + 128]
            w_f2 = wm[0:64, _W_FW2:_W_FW2 + 128]
            gb2b4 = wm[0:128, _W_GB2:_W_GB2 + 512]
            b_f1 = wm[0:64, _B_F1:_B_F1 + 2].bitcast(f32)
            b_g1 = wm[0:128, _B_G1:_B_G1 + 2].bitcast(f32)
            b_f2 = wm[0:128, _B_F2:_B_F2 + 2].bitcast(f32)

            h1f = apool.tile([D_IN, M], f16, name="h1f")
            h1g = apool.tile([D_HID, M], f16, name="h1g")
            # gx[p, 128*(2b+g) + h] = g(x)[sender 256b + 2p + g][h] in fp8
            gx = apool.tile([128, M], f8, name="gx")
            outTa = apool.tile([128, M // 2], f16, name="outTa")
            outTb = apool.tile([128, M // 2], f16, name="outTb")
            pout = pout_pool.tile([128, M], f32, name="pout")

            # dense warm-up matmuls until the preamble data lands: the HAM
            # clock boost (1.2 -> 2.4 GHz) arrives ~12us after SUSTAINED PE
            # activity begins, so keep the PE busy from the earliest moment.
            # One activation hoists the lazy ~1.3us ACT table load off the
            # h1g critical path. memset on Pool (fast, idle).
            warm2 = apool.tile([1, 1], f32, name="warm2")
            nc.gpsimd.memset(scratch, 0)
            for _ in range(2):
                psw = pg_pool.tile([128, 512], f32, tag="g", name="psw")
                nc.tensor.matmul(psw, scratch[:, 0:128], scratch,
                                 start=True, stop=True)
            nc.scalar.activation(warm2, scratch[0:1, 0:1], AF.Relu, bias=0.0)

            # token chunk k (tokens 512k..512k+511): xT partitions 64a..64a+64,
            # columns 512c2..512c2+512 with (a, c2) = divmod(k, 2)
            def mm_h1g(k):
                a, c2 = divmod(k, 2)
                w_g1 = wm[64 * a:64 * a + 64, _W_WG1:_W_WG1 + 128]
                rhs = xT[64 * a:64 * a + 64, 512 * c2:512 * c2 + 512]
                psg = pg_pool.tile([128, 512], f32, tag="g", name="psg")
                nc.tensor.matmul(psg, w_g1, rhs, start=True, stop=True)
                nc.scalar.activation(h1g[:, ts(k, 512)], psg, AF.Relu, bias=b_g1)

            def gx_chunk(k):
                # h1g chunk k -> gx tiles 4k..4k+3 (pair blocks 2k, 2k+1).
                # Stationary stride-2 token slices produce the pair-interleaved
                # sender order matching the edge DMA layout.
                psx = pwork_pool.tile([128, 512], f32, tag="w", name="psx")
                for t in range(4):
                    b, g = divmod(t, 2)
                    toks = h1g[:, 512 * k + 256 * b + g: 512 * k + 256 * (b + 1): 2]
                    nc.tensor.matmul(psx[:, ts(t, 128)], toks, w_g2,
                                     start=True, stop=True)
                nc.vector.tensor_add(gx[:, ts(k, 512)], psx, gb2b4)

            def h1f_chunk(k):
                a, c2 = divmod(k, 2)
                w_f1 = wm[64 * a:64 * a + 64, _W_FW1:_W_FW1 + 64]
                rhs = xT[64 * a:64 * a + 64, 512 * c2:512 * c2 + 512]
                psf = pwork_pool.tile([64, 512], f32, tag="w", name="psf")
                nc.tensor.matmul(psf, w_f1, rhs, start=True, stop=True)
                nc.scalar.activation(h1f[:, ts(k, 512)], psf, AF.Relu, bias=b_f1)

            def sd_chunk(k):
                # self-dynamics opens pout's accumulation group
                nc.tensor.matmul(pout[:, ts(k, 512)], w_f2, h1f[:, ts(k, 512)],
                                 start=True, stop=False)

            def edge_block(b):
                et = ets[b]
                lhsT = gx[:, 256 * b:256 * b + 256].rearrange(
                    "p (g h) -> p g h", g=2
                )
                for c in range(NCH):
                    nc.tensor.matmul(
                        pout[:, ts(c, 512)], lhsT, et[:, :, ts(c, 512)],
                        start=False, stop=(b == NPAIR - 1),
                        perf_mode=DR,
                    )

            # pre-boost PE work, minimal and stall-free: h1g chunks staggered
            # one ahead of their gx consumers (relu_k trails h1g_k on ACT by
            # ~0.7us); the ACT-independent h1f matmuls plug the relu-wait gaps
            mm_h1g(0)
            mm_h1g(1)
            h1f_chunk(0)
            gx_chunk(0)
            mm_h1g(2)
            h1f_chunk(1)
            gx_chunk(1)
            mm_h1g(3)
            h1f_chunk(2)
            gx_chunk(2)
            h1f_chunk(3)
            gx_chunk(3)
            for k in range(NCH):
                sd_chunk(k)
            for b in range(NPAIR):
                edge_block(b)

            # tail: per-chunk PSUM->SBUF bias-copy (ACT/DVE alternating,
            # pipelined behind the last edge block's chunk matmuls) + stores
            for c in range(NCH):
                ot = outTa if c < 2 else outTb
                src = pout[:, ts(c, 512)]
                dst = ot[:, ts(c % 2, 512)]
                if c % 2 == 0:
                    nc.scalar.activation(dst, src, AF.Identity, bias=b_f2)
                else:
                    nc.vector.tensor_scalar_add(dst, src, b_f2)
                if c % 2 == 1:
                    nc.sync.dma_start(out=outT_d[:, ts(c // 2, 1024)], in_=ot)
    nc.compile()
    return nc


def _get_nc():
    if "nc" not in _NC_CACHE:
        _NC_CACHE["nc"] = _build()
    return _NC_CACHE["nc"]


def _prep_in_maps(inputs):
    import ml_dtypes

    f8 = ml_dtypes.float8_e4m3

    x = np.asarray(inputs["x"], dtype=np.float32)
    edge = np.asarray(inputs["edge"], dtype=np.float32)
    f_w1 = np.asarray(inputs["f_w1"], dtype=np.float32)
    f_b1 = np.asarray(inputs["f_b1"], dtype=np.float32)
    f_w2 = np.asarray(inputs["f_w2"], dtype=np.float32)
    f_b2 = np.asarray(inputs["f_b2"], dtype=np.float32)
    g_w1 = np.asarray(inputs["g_w1"], dtype=np.float32)
    g_b1 = np.asarray(inputs["g_b1"], dtype=np.float32)
    g_w2 = np.asarray(inputs["g_w2"], dtype=np.float32)
    g_b2 = np.asarray(inputs["g_b2"], dtype=np.float32)

    # cat(x, x) @ g_w1 == x @ (g_w1[:64] + g_w1[64:])
    wg1 = g_w1[:D_IN] + g_w1[D_IN:]

    # x[n].T packed [128, 1024]: xT2[64a + k, t] = x[n, 1024a + t, k]
    xT = np.transpose(x, (0, 2, 1)).astype(np.float16)       # [8, 64, 2048]
    xT2 = np.concatenate([xT[:, :, :1024], xT[:, :, 1024:]], axis=1)

    def f32_pairs(v):
        # fp32 values bit-packed into consecutive fp16 slots
        return v.astype("<f4").view("<f2")

    wm = np.zeros((128, WM_W), dtype=np.float16)
    for r in (slice(0, 64), slice(64, 128)):  # duplicate for partition-64 rhs
        wm[r, _W_FW1:_W_FW1 + 64] = f_w1.astype(np.float16)
        wm[r, _W_FW2:_W_FW2 + 128] = f_w2.astype(np.float16)
        wm[r, _W_WG1:_W_WG1 + 128] = wg1.astype(np.float16)
    wm[:, _W_GW2:_W_GW2 + 128] = g_w2.astype(np.float16)
    wm[:, _W_GB2:_W_GB2 + 512] = np.tile(
        g_b2[None, :].astype(np.float16), (128, 4))
    wm[0:64, _B_F1:_B_F1 + 2] = f32_pairs(f_b1).reshape(64, 2)
    wm[0:128, _B_G1:_B_G1 + 2] = f32_pairs(g_b1).reshape(128, 2)
    wm[0:128, _B_F2:_B_F2 + 2] = f32_pairs(f_b2).reshape(128, 2)

    # edge rows stay in natural order; the pair interleave is expressed by
    # the DMA access pattern, so the host just casts to fp8.
    edge8 = edge.astype(f8)
    in_maps = [
        {
            "wm": wm,
            "xT": np.ascontiguousarray(xT2[n]),
            "edge": np.ascontiguousarray(edge8[n]),
        }
        for n in range(N_BATCH)
    ]
    return in_maps


def run(inputs, trace=False, **kw):
    """Run on 8 cores; returns (out [8, 2048, 128] fp32, BassKernelResults)."""
    from concourse.bass_utils import run_bass_kernel_spmd

    nc = _get_nc()
    in_maps = _prep_in_maps(inputs)
    res = run_bass_kernel_spmd(nc, in_maps, list(range(NCORES)), trace=trace, **kw)
    outT = np.stack([np.asarray(res.results[n]["outT"]) for n in range(N_BATCH)])
    out = np.ascontiguousarray(np.transpose(outT, (0, 2, 1)))  # [8, 2048, 128]
    return out.astype(np.float32), res


def kernel(**inputs):
    out, _ = run(inputs, trace=False)
    return out


# revision 28
# speedup vs baseline: 1.1533x; 1.1533x over previous
"""Trainium2 Bass kernel for nn_Node_GCN: out[n] = f(x[n]) + edge[n]^T @ g(cat(x,x)[n]).

Sharding: data-parallel over the batch dim N=8, one batch per NeuronCore.
Per core the dominant cost is streaming edge[n] from HBM once. edge and gx are
carried in fp8-e4m3 (fp32 PSUM accumulation), halving HBM traffic vs fp16 and
running DoubleRow matmuls (two 128-row contraction groups per instruction, 1
output column/cycle -> 2x fp16 FLOP rate). Edge rows are pair-interleaved via
the DMA access pattern (partition p holds rows 2p, 2p+1 of each 256-row block)
so descriptors are 4KB; the matching gx sender order comes from stride-2
stationary slices of h1g.

Timing model (measured): the NeuronCore runs at 1.2GHz until a HAM boost
event ~12us after the first PE matmul, then 2.4GHz. So: one tiny matmul
starts the boost timer immediately; everything else minimizes pre-boost PE
cycles. All loads ride one Sync-queue stream ordered [blob, edge x4] (a
parallel queue gets starved). Self-dynamics accumulates into pout (opens the
accumulation group) so the tail is just per-chunk PSUM->SBUF bias-copies
(ACT/DVE alternating) + two fp16 stores.

The device computes outT[n] = [h, j]; the host transposes to [j, h] while
unsharding.
"""

import numpy as np

D_IN = 64
D_HID = 128
M = 2048          # nodes per batch
N_BATCH = 8
NCORES = 8
NPAIR = M // 256  # 8 edge pair-blocks of 256 sender rows

# fp16 weights mini-blob [128, WM_W] (first DMA, unlocks the MLP chain
# early) + fp16 xT blob [128, 1024]. Rows 64:128 of the weight section
# duplicate rows 0:64 for the K=64 matmuls whose rhs lives on partitions
# 64:128 (xT packed [128, 1024]). fp32 biases are bit-packed as fp16 pairs.
_W_FW1 = 0          # f_w1 [64, 64]
_W_FW2 = 64         # f_w2 [64, 128]
_W_WG1 = 192        # wg1  [64, 128]  (= g_w1[:64] + g_w1[64:])
_W_GW2 = 320        # g_w2 [128, 128]
_W_GB2 = 448        # g_b2 broadcast rows, tiled 4x along free dim [128, 512]
_B_F1 = 960         # f_b1 [64, 1] fp32 as 2 fp16 slots
_B_G1 = 962         # g_b1 [128, 1] fp32 as 2 fp16 slots
_B_F2 = 964         # f_b2 [128, 1] fp32 as 2 fp16 slots
WM_W = 968

_NC_CACHE = {}


def _build():
    import concourse.bacc as bacc
    import concourse.mybir as mybir
    from concourse.tile import TileContext
    from concourse.bass import ts

    f32 = mybir.dt.float32
    f16 = mybir.dt.float16
    f8 = mybir.dt.float8e4
    AF = mybir.ActivationFunctionType
    DR = mybir.MatmulPerfMode.DoubleRow

    nc = bacc.Bacc()
    edge_d = nc.declare_dram_parameter("edge", [M, M], f8, isOutput=False)
    wm_d = nc.declare_dram_parameter("wm", [128, WM_W], f16, isOutput=False)
    xT_d = nc.declare_dram_parameter("xT", [128, M // 2], f16, isOutput=False)
    outT_d = nc.declare_dram_parameter("outT", [D_HID, M], f16, isOutput=True)

    NCH = M // 512   # 4 chunks of 512 for wide matmuls

    with TileContext(nc) as tc:
        with (
            tc.tile_pool(name="const", bufs=1) as cpool,
            tc.tile_pool(name="acts", bufs=1) as apool,
            tc.tile_pool(name="edgep", bufs=NPAIR) as epool,
            tc.tile_pool(name="pout", bufs=1, space="PSUM") as pout_pool,
            tc.tile_pool(name="pg", bufs=2, space="PSUM") as pg_pool,
            tc.tile_pool(name="pwork", bufs=2, space="PSUM") as pwork_pool,
        ):
            wm = cpool.tile([128, WM_W], f16, name="wm")
            xT = cpool.tile([128, M // 2], f16, name="xT")
            scratch = apool.tile([128, 512], f16, name="scratch")

            # Preamble loads first on the Sync queue (small loads lose
            # DMA-engine arbitration beside the edge stream): weights mini
            # blob, then xT, then the edge stream as 4 x 1MB, each covering
            # pair-blocks 2d, 2d+1.
            nc.sync.dma_start(out=wm, in_=wm_d[:])
            nc.sync.dma_start(out=xT[:, 0:512], in_=xT_d[:, 0:512])
            nc.sync.dma_start(out=xT[:, 512:1024], in_=xT_d[:, 512:1024])
            ets = []
            for b in range(NPAIR):
                et = epool.tile([128, 2, M], f8, tag="e", name=f"et{b}")
                src = edge_d[256 * b:256 * b + 256, :].rearrange(
                    "(p g) j -> p g j", g=2
                )
                nc.sync.dma_start(out=et, in_=src)
                ets.append(et)

            w_g2 = wm[0:128, _W_GW2:_W_GW2 + 128]
            w_f2 = wm[0:64, _W_FW2:_W_FW2 + 128]
            gb2b4 = wm[0:128, _W_GB2:_W_GB2 + 512]
            b_f1 = wm[0:64, _B_F1:_B_F1 + 2].bitcast(f32)
            b_g1 = wm[0:128, _B_G1:_B_G1 + 2].bitcast(f32)
            b_f2 = wm[0:128, _B_F2:_B_F2 + 2].bitcast(f32)

            h1f = apool.tile([D_IN, M], f16, name="h1f")
            h1g = apool.tile([D_HID, M], f16, name="h1g")
            # gx[p, 128*(2b+g) + h] = g(x)[sender 256b + 2p + g][h] in fp8
            gx = apool.tile([128, M], f8, name="gx")
            outTa = apool.tile([128, M // 2], f16, name="outTa")
            outTb = apool.tile([128, M // 2], f16, name="outTb")
            pout = pout_pool.tile([128, M], f32, name="pout")

            # dense warm-up matmuls until the preamble data lands: the HAM
            # clock boost (1.2 -> 2.4 GHz) arrives ~12us after SUSTAINED PE
            # activity begins, so keep the PE busy from the earliest moment.
            # One activation hoists the lazy ~1.3us ACT table load off the
            # h1g critical path. memset on Pool (fast, idle).
            warm2 = apool.tile([1, 1], f32, name="warm2")
            nc.gpsimd.memset(scratch, 0)
            for _ in range(2):
                psw = pg_pool.tile([128, 512], f32, tag="g", name="psw")
                nc.tensor.matmul(psw, scratch[:, 0:128], scratch,
                                 start=True, stop=True)
            nc.scalar.activation(warm2, scratch[0:1, 0:1], AF.Relu, bias=0.0)

            # token chunk k (tokens 512k..512k+511): xT partitions 64a..64a+64,
            # columns 512c2..512c2+512 with (a, c2) = divmod(k, 2)
            def mm_h1g(k):
                a, c2 = divmod(k, 2)
                w_g1 = wm[64 * a:64 * a + 64, _W_WG1:_W_WG1 + 128]
                rhs = xT[64 * a:64 * a + 64, 512 * c2:512 * c2 + 512]
                psg = pg_pool.tile([128, 512], f32, tag="g", name="psg")
                nc.tensor.matmul(psg, w_g1, rhs, start=True, stop=True)
                nc.scalar.activation(h1g[:, ts(k, 512)], psg, AF.Relu, bias=b_g1)

            def gx_chunk(k):
                # h1g chunk k -> gx tiles 4k..4k+3 (pair blocks 2k, 2k+1).
                # Stationary stride-2 token slices produce the pair-interleaved
                # sender order matching the edge DMA layout.
                psx = pwork_pool.tile([128, 512], f32, tag="w", name="psx")
                for t in range(4):
                    b, g = divmod(t, 2)
                    toks = h1g[:, 512 * k + 256 * b + g: 512 * k + 256 * (b + 1): 2]
                    nc.tensor.matmul(psx[:, ts(t, 128)], toks, w_g2,
                                     start=True, stop=True)
                nc.vector.tensor_add(gx[:, ts(k, 512)], psx, gb2b4)

            def h1f_chunk(k):
                a, c2 = divmod(k, 2)
                w_f1 = wm[64 * a:64 * a + 64, _W_FW1:_W_FW1 + 64]
                rhs = xT[64 * a:64 * a + 64, 512 * c2:512 * c2 + 512]
                psf = pwork_pool.tile([64, 512], f32, tag="w", name="psf")
                nc.tensor.matmul(psf, w_f1, rhs, start=True, stop=True)
                nc.scalar.activation(h1f[:, ts(k, 512)], psf, AF.Relu, bias=b_f1)

            def sd_chunk(k):
                # self-dynamics opens pout's accumulation group
                nc.tensor.matmul(pout[:, ts(k, 512)], w_f2, h1f[:, ts(k, 512)],
                                 start=True, stop=False)

            def edge_block(b):
                et = ets[b]
                lhsT = gx[:, 256 * b:256 * b + 256].rearrange(
                    "p (g h) -> p g h", g=2
                )
                for c in range(NCH):
                    nc.tensor.matmul(
                        pout[:, ts(c, 512)], lhsT, et[:, :, ts(c, 512)],
                        start=False, stop=(b == NPAIR - 1),
                        perf_mode=DR,
                    )

            # pre-boost PE work, minimal and stall-free: h1g chunks staggered
            # one ahead of their gx consumers (relu_k trails h1g_k on ACT by
            # ~0.7us); the ACT-independent h1f matmuls plug the relu-wait gaps
            mm_h1g(0)
            mm_h1g(1)
            h1f_chunk(0)
            gx_chunk(0)
            mm_h1g(2)
            h1f_chunk(1)
            gx_chunk(1)
            mm_h1g(3)
            h1f_chunk(2)
            gx_chunk(2)
            h1f_chunk(3)
            gx_chunk(3)
            for k in range(NCH):
                sd_chunk(k)
            for b in range(NPAIR):
                edge_block(b)

            # tail: per-chunk PSUM->SBUF bias-copy (ACT/DVE alternating,
            # pipelined behind the last edge block's chunk matmuls) + stores
            for c in range(NCH):
                ot = outTa if c < 2 else outTb
                src = pout[:, ts(c, 512)]
                dst = ot[:, ts(c % 2, 512)]
                if c % 2 == 0:
                    nc.scalar.activation(dst, src, AF.Identity, bias=b_f2)
                else:
                    nc.vector.tensor_scalar_add(dst, src, b_f2)
                if c % 2 == 1:
                    nc.sync.dma_start(out=outT_d[:, ts(c // 2, 1024)], in_=ot)
    nc.compile()
    return nc


def _get_nc():
    if "nc" not in _NC_CACHE:
        _NC_CACHE["nc"] = _build()
    return _NC_CACHE["nc"]


def _prep_in_maps(inputs):
    import ml_dtypes

    f8 = ml_dtypes.float8_e4m3

    x = np.asarray(inputs["x"], dtype=np.float32)
    edge = np.asarray(inputs["edge"], dtype=np.float32)
    f_w1 = np.asarray(inputs["f_w1"], dtype=np.float32)
    f_b1 = np.asarray(inputs["f_b1"], dtype=np.float32)
    f_w2 = np.asarray(inputs["f_w2"], dtype=np.float32)
    f_b2 = np.asarray(inputs["f_b2"], dtype=np.float32)
    g_w1 = np.asarray(inputs["g_w1"], dtype=np.float32)
    g_b1 = np.asarray(inputs["g_b1"], dtype=np.float32)
    g_w2 = np.asarray(inputs["g_w2"], dtype=np.float32)
    g_b2 = np.asarray(inputs["g_b2"], dtype=np.float32)

    # cat(x, x) @ g_w1 == x @ (g_w1[:64] + g_w1[64:])
    wg1 = g_w1[:D_IN] + g_w1[D_IN:]

    # x[n].T packed [128, 1024]: xT2[64a + k, t] = x[n, 1024a + t, k]
    xT = np.transpose(x, (0, 2, 1)).astype(np.float16)       # [8, 64, 2048]
    xT2 = np.concatenate([xT[:, :, :1024], xT[:, :, 1024:]], axis=1)

    def f32_pairs(v):
        # fp32 values bit-packed into consecutive fp16 slots
        return v.astype("<f4").view("<f2")

    wm = np.zeros((128, WM_W), dtype=np.float16)
    for r in (slice(0, 64), slice(64, 128)):  # duplicate for partition-64 rhs
        wm[r, _W_FW1:_W_FW1 + 64] = f_w1.astype(np.float16)
        wm[r, _W_FW2:_W_FW2 + 128] = f_w2.astype(np.float16)
        wm[r, _W_WG1:_W_WG1 + 128] = wg1.astype(np.float16)
    wm[:, _W_GW2:_W_GW2 + 128] = g_w2.astype(np.float16)
    wm[:, _W_GB2:_W_GB2 + 512] = np.tile(
        g_b2[None, :].astype(np.float16), (128, 4))
    wm[0:64, _B_F1:_B_F1 + 2] = f32_pairs(f_b1).reshape(64, 2)
    wm[0:128, _B_G1:_B_G1 + 2] = f32_pairs(g_b1).reshape(128, 2)
    wm[0:128, _B_F2:_B_F2 + 2] = f32_pairs(f_b2).reshape(128, 2)

    # edge rows stay in natural order; the pair interleave is expressed by
    # the DMA access pattern, so the host just casts to fp8.
    edge8 = edge.astype(f8)
    in_maps = [
        {
            "wm": wm,
            "xT": np.ascontiguousarray(xT2[n]),
            "edge": np.ascontiguousarray(edge8[n]),
        }
        for n in range(N_BATCH)
    ]
    return in_maps


def run(inputs, trace=False, **kw):
    """Run on 8 cores; returns (out [8, 2048, 128] fp32, BassKernelResults)."""
    from concourse.bass_utils import run_bass_kernel_spmd

    nc = _get_nc()
    in_maps = _prep_in_maps(inputs)
    res = run_bass_kernel_spmd(nc, in_maps, list(range(NCORES)), trace=trace, **kw)
    outT = np.stack([np.asarray(res.results[n]["outT"]) for n in range(N_BATCH)])
    out = np.ascontiguousarray(np.transpose(outT, (0, 2, 1)))  # [8, 2048, 128]
    return out.astype(np.float32), res


def kernel(**inputs):
    out, _ = run(inputs, trace=False)
    return out


# revision 29
# speedup vs baseline: 1.1873x; 1.0295x over previous
"""Trainium2 Bass kernel for nn_Node_GCN: out[n] = f(x[n]) + edge[n]^T @ g(cat(x,x)[n]).

Sharding: data-parallel over the batch dim N=8, one batch per NeuronCore.
Per core the dominant cost is streaming edge[n] from HBM once. edge and gx are
carried in fp8-e4m3 (fp32 PSUM accumulation), halving HBM traffic vs fp16 and
running DoubleRow matmuls (two 128-row contraction groups per instruction, 1
output column/cycle -> 2x fp16 FLOP rate). Edge rows are pair-interleaved via
the DMA access pattern (partition p holds rows 2p, 2p+1 of each 256-row block)
so descriptors are 4KB; the matching gx sender order comes from stride-2
stationary slices of h1g.

Timing model (measured): the NeuronCore runs at 1.2GHz until a HAM boost
event ~12us after the first PE matmul, then 2.4GHz. So: one tiny matmul
starts the boost timer immediately; everything else minimizes pre-boost PE
cycles. All loads ride one Sync-queue stream ordered [blob, edge x4] (a
parallel queue gets starved). Self-dynamics accumulates into pout (opens the
accumulation group) so the tail is just per-chunk PSUM->SBUF bias-copies
(ACT/DVE alternating) + two fp16 stores.

The device computes outT[n] = [h, j]; the host transposes to [j, h] while
unsharding.
"""

import numpy as np

D_IN = 64
D_HID = 128
M = 2048          # nodes per batch
N_BATCH = 8
NCORES = 8
NPAIR = M // 256  # 8 edge pair-blocks of 256 sender rows

# fp16 weights mini-blob [128, WM_W] (first DMA, unlocks the MLP chain
# early) + fp16 xT blob [128, 1024]. Rows 64:128 of the weight section
# duplicate rows 0:64 for the K=64 matmuls whose rhs lives on partitions
# 64:128 (xT packed [128, 1024]). fp32 biases are bit-packed as fp16 pairs.
_W_FW1 = 0          # f_w1 [64, 64]
_W_FW2 = 64         # f_w2 [64, 128]
_W_WG1 = 192        # wg1  [64, 128]  (= g_w1[:64] + g_w1[64:])
_W_GW2 = 320        # g_w2 [128, 128]
_W_GB2 = 448        # g_b2 broadcast rows, tiled 4x along free dim [128, 512]
_B_F1 = 960         # f_b1 [64, 1] fp32 as 2 fp16 slots
_B_G1 = 962         # g_b1 [128, 1] fp32 as 2 fp16 slots
_B_F2 = 964         # f_b2 [128, 1] fp32 as 2 fp16 slots
WM_W = 968

_NC_CACHE = {}


def _build():
    import concourse.bacc as bacc
    import concourse.mybir as mybir
    from concourse.tile import TileContext
    from concourse.bass import ts

    f32 = mybir.dt.float32
    f16 = mybir.dt.float16
    f8 = mybir.dt.float8e4
    AF = mybir.ActivationFunctionType
    DR = mybir.MatmulPerfMode.DoubleRow

    nc = bacc.Bacc()
    # All DMAs ride the SP (Sync) hardware DGE ring; drop the unused
    # Activation-HWDGE and Pool-SWDGE queue declarations so the NEFF epilogue
    # has fewer per-queue semaphores to reset (the ~50 individual per-engine
    # resets cost ~6us of teardown).
    nc.m.queues = [q for q in nc.m.queues if q.name == "qSPDynamicHW"]
    edge_d = nc.declare_dram_parameter("edge", [M, M], f8, isOutput=False)
    wm_d = nc.declare_dram_parameter("wm", [128, WM_W], f16, isOutput=False)
    xT_d = nc.declare_dram_parameter("xT", [128, M // 2], f16, isOutput=False)
    outT_d = nc.declare_dram_parameter("outT", [D_HID, M], f16, isOutput=True)

    NCH = M // 512   # 4 chunks of 512 for wide matmuls

    with TileContext(nc) as tc:
        with (
            tc.tile_pool(name="const", bufs=1) as cpool,
            tc.tile_pool(name="acts", bufs=1) as apool,
            tc.tile_pool(name="edgep", bufs=NPAIR) as epool,
            tc.tile_pool(name="pout", bufs=1, space="PSUM") as pout_pool,
            tc.tile_pool(name="pg", bufs=2, space="PSUM") as pg_pool,
            tc.tile_pool(name="pwork", bufs=2, space="PSUM") as pwork_pool,
        ):
            wm = cpool.tile([128, WM_W], f16, name="wm")
            xT = cpool.tile([128, M // 2], f16, name="xT")
            scratch = apool.tile([128, 512], f16, name="scratch")

            # Preamble loads first on the Sync queue (small loads lose
            # DMA-engine arbitration beside the edge stream): weights mini
            # blob, then xT, then the edge stream as 4 x 1MB, each covering
            # pair-blocks 2d, 2d+1.
            nc.sync.dma_start(out=wm, in_=wm_d[:])
            nc.sync.dma_start(out=xT[:, 0:512], in_=xT_d[:, 0:512])
            nc.sync.dma_start(out=xT[:, 512:1024], in_=xT_d[:, 512:1024])
            ets = []
            for b in range(NPAIR):
                et = epool.tile([128, 2, M], f8, tag="e", name=f"et{b}")
                src = edge_d[256 * b:256 * b + 256, :].rearrange(
                    "(p g) j -> p g j", g=2
                )
                nc.sync.dma_start(out=et, in_=src)
                ets.append(et)

            w_g2 = wm[0:128, _W_GW2:_W_GW2 + 128]
            w_f2 = wm[0:64, _W_FW2:_W_FW2 + 128]
            gb2b4 = wm[0:128, _W_GB2:_W_GB2 + 512]
            b_f1 = wm[0:64, _B_F1:_B_F1 + 2].bitcast(f32)
            b_g1 = wm[0:128, _B_G1:_B_G1 + 2].bitcast(f32)
            b_f2 = wm[0:128, _B_F2:_B_F2 + 2].bitcast(f32)

            h1f = apool.tile([D_IN, M], f16, name="h1f")
            h1g = apool.tile([D_HID, M], f16, name="h1g")
            # gx[p, 128*(2b+g) + h] = g(x)[sender 256b + 2p + g][h] in fp8
            gx = apool.tile([128, M], f8, name="gx")
            outTa = apool.tile([128, M // 2], f16, name="outTa")
            outTb = apool.tile([128, M // 2], f16, name="outTb")
            pout = pout_pool.tile([128, M], f32, name="pout")

            # dense warm-up matmuls until the preamble data lands: the HAM
            # clock boost (1.2 -> 2.4 GHz) arrives ~12us after SUSTAINED PE
            # activity begins, so keep the PE busy from the earliest moment.
            # One activation hoists the lazy ~1.3us ACT table load off the
            # h1g critical path. memset on Pool (fast, idle).
            warm2 = apool.tile([1, 1], f32, name="warm2")
            nc.gpsimd.memset(scratch, 0)
            for _ in range(2):
                psw = pg_pool.tile([128, 512], f32, tag="g", name="psw")
                nc.tensor.matmul(psw, scratch[:, 0:128], scratch,
                                 start=True, stop=True)
            nc.scalar.activation(warm2, scratch[0:1, 0:1], AF.Relu, bias=0.0)

            # token chunk k (tokens 512k..512k+511): xT partitions 64a..64a+64,
            # columns 512c2..512c2+512 with (a, c2) = divmod(k, 2)
            def mm_h1g(k):
                a, c2 = divmod(k, 2)
                w_g1 = wm[64 * a:64 * a + 64, _W_WG1:_W_WG1 + 128]
                rhs = xT[64 * a:64 * a + 64, 512 * c2:512 * c2 + 512]
                psg = pg_pool.tile([128, 512], f32, tag="g", name="psg")
                nc.tensor.matmul(psg, w_g1, rhs, start=True, stop=True)
                nc.scalar.activation(h1g[:, ts(k, 512)], psg, AF.Relu, bias=b_g1)

            def gx_chunk(k):
                # h1g chunk k -> gx tiles 4k..4k+3 (pair blocks 2k, 2k+1).
                # Stationary stride-2 token slices produce the pair-interleaved
                # sender order matching the edge DMA layout.
                psx = pwork_pool.tile([128, 512], f32, tag="w", name="psx")
                for t in range(4):
                    b, g = divmod(t, 2)
                    toks = h1g[:, 512 * k + 256 * b + g: 512 * k + 256 * (b + 1): 2]
                    nc.tensor.matmul(psx[:, ts(t, 128)], toks, w_g2,
                                     start=True, stop=True)
                nc.vector.tensor_add(gx[:, ts(k, 512)], psx, gb2b4)

            def h1f_chunk(k):
                a, c2 = divmod(k, 2)
                w_f1 = wm[64 * a:64 * a + 64, _W_FW1:_W_FW1 + 64]
                rhs = xT[64 * a:64 * a + 64, 512 * c2:512 * c2 + 512]
                psf = pwork_pool.tile([64, 512], f32, tag="w", name="psf")
                nc.tensor.matmul(psf, w_f1, rhs, start=True, stop=True)
                nc.scalar.activation(h1f[:, ts(k, 512)], psf, AF.Relu, bias=b_f1)

            def sd_chunk(k):
                # self-dynamics opens pout's accumulation group
                nc.tensor.matmul(pout[:, ts(k, 512)], w_f2, h1f[:, ts(k, 512)],
                                 start=True, stop=False)

            def edge_block(b):
                et = ets[b]
                lhsT = gx[:, 256 * b:256 * b + 256].rearrange(
                    "p (g h) -> p g h", g=2
                )
                for c in range(NCH):
                    nc.tensor.matmul(
                        pout[:, ts(c, 512)], lhsT, et[:, :, ts(c, 512)],
                        start=False, stop=(b == NPAIR - 1),
                        perf_mode=DR,
                    )

            # pre-boost PE work, minimal and stall-free: h1g chunks staggered
            # one ahead of their gx consumers (relu_k trails h1g_k on ACT by
            # ~0.7us); the ACT-independent h1f matmuls plug the relu-wait gaps
            mm_h1g(0)
            mm_h1g(1)
            h1f_chunk(0)
            gx_chunk(0)
            mm_h1g(2)
            h1f_chunk(1)
            gx_chunk(1)
            mm_h1g(3)
            h1f_chunk(2)
            gx_chunk(2)
            h1f_chunk(3)
            gx_chunk(3)
            for k in range(NCH):
                sd_chunk(k)
            for b in range(NPAIR):
                edge_block(b)

            # tail: per-chunk PSUM->SBUF bias-copy (ACT/DVE alternating,
            # pipelined behind the last edge block's chunk matmuls) + stores
            for c in range(NCH):
                ot = outTa if c < 2 else outTb
                src = pout[:, ts(c, 512)]
                dst = ot[:, ts(c % 2, 512)]
                if c % 2 == 0:
                    nc.scalar.activation(dst, src, AF.Identity, bias=b_f2)
                else:
                    nc.vector.tensor_scalar_add(dst, src, b_f2)
                if c % 2 == 1:
                    nc.sync.dma_start(out=outT_d[:, ts(c // 2, 1024)], in_=ot)
    nc.compile()
    return nc


def _get_nc():
    if "nc" not in _NC_CACHE:
        _NC_CACHE["nc"] = _build()
    return _NC_CACHE["nc"]


def _prep_in_maps(inputs):
    import ml_dtypes

    f8 = ml_dtypes.float8_e4m3

    x = np.asarray(inputs["x"], dtype=np.float32)
    edge = np.asarray(inputs["edge"], dtype=np.float32)
    f_w1 = np.asarray(inputs["f_w1"], dtype=np.float32)
    f_b1 = np.asarray(inputs["f_b1"], dtype=np.float32)
    f_w2 = np.asarray(inputs["f_w2"], dtype=np.float32)
    f_b2 = np.asarray(inputs["f_b2"], dtype=np.float32)
    g_w1 = np.asarray(inputs["g_w1"], dtype=np.float32)
    g_b1 = np.asarray(inputs["g_b1"], dtype=np.float32)
    g_w2 = np.asarray(inputs["g_w2"], dtype=np.float32)
    g_b2 = np.asarray(inputs["g_b2"], dtype=np.float32)

    # cat(x, x) @ g_w1 == x @ (g_w1[:64] + g_w1[64:])
    wg1 = g_w1[:D_IN] + g_w1[D_IN:]

    # x[n].T packed [128, 1024]: xT2[64a + k, t] = x[n, 1024a + t, k]
    xT = np.transpose(x, (0, 2, 1)).astype(np.float16)       # [8, 64, 2048]
    xT2 = np.concatenate([xT[:, :, :1024], xT[:, :, 1024:]], axis=1)

    def f32_pairs(v):
        # fp32 values bit-packed into consecutive fp16 slots
        return v.astype("<f4").view("<f2")

    wm = np.zeros((128, WM_W), dtype=np.float16)
    for r in (slice(0, 64), slice(64, 128)):  # duplicate for partition-64 rhs
        wm[r, _W_FW1:_W_FW1 + 64] = f_w1.astype(np.float16)
        wm[r, _W_FW2:_W_FW2 + 128] = f_w2.astype(np.float16)
        wm[r, _W_WG1:_W_WG1 + 128] = wg1.astype(np.float16)
    wm[:, _W_GW2:_W_GW2 + 128] = g_w2.astype(np.float16)
    wm[:, _W_GB2:_W_GB2 + 512] = np.tile(
        g_b2[None, :].astype(np.float16), (128, 4))
    wm[0:64, _B_F1:_B_F1 + 2] = f32_pairs(f_b1).reshape(64, 2)
    wm[0:128, _B_G1:_B_G1 + 2] = f32_pairs(g_b1).reshape(128, 2)
    wm[0:128, _B_F2:_B_F2 + 2] = f32_pairs(f_b2).reshape(128, 2)

    # edge rows stay in natural order; the pair interleave is expressed by
    # the DMA access pattern, so the host just casts to fp8.
    edge8 = edge.astype(f8)
    in_maps = [
        {
            "wm": wm,
            "xT": np.ascontiguousarray(xT2[n]),
            "edge": np.ascontiguousarray(edge8[n]),
        }
        for n in range(N_BATCH)
    ]
    return in_maps


def run(inputs, trace=False, **kw):
    """Run on 8 cores; returns (out [8, 2048, 128] fp32, BassKernelResults)."""
    from concourse.bass_utils import run_bass_kernel_spmd

    nc = _get_nc()
    in_maps = _prep_in_maps(inputs)
    res = run_bass_kernel_spmd(nc, in_maps, list(range(NCORES)), trace=trace, **kw)
    outT = np.stack([np.asarray(res.results[n]["outT"]) for n in range(N_BATCH)])
    out = np.ascontiguousarray(np.transpose(outT, (0, 2, 1)))  # [8, 2048, 128]
    return out.astype(np.float32), res


def kernel(**inputs):
    out, _ = run(inputs, trace=False)
    return out
